# revision 12
# baseline (speedup 1.0000x reference)
"""Trainium2 Bass kernel for nn_CAPMemory (camera-aware proxy memory loss).

Strategy (8 NeuronCores, SPMD, no collectives):
  - Shard the 64000x256 proxy/center table over P: core k owns centers rows
    [8000k, 8000(k+1)) = 1000 labels x 8 cams (contiguous), transposed on the
    host to [256, 8000] for direct use as the matmul moving operand.
  - Batch rows (512) are replicated on every core, permuted so rows are sorted
    by camera id (intra-loss camera selection becomes strided-AP reductions on
    contiguous partition ranges).
  - Each core computes its [512, 8000] slice of the similarity matrix with the
    PE (feats normalized + transposed on device), then reduces it on device to
    small per-core outputs:
      cand [512, 32*8]  top-8 values of each 250-column chunk (DVE InstMax)
      mrow [512]        max_l sims_intra (camera-selected, strided AP)
      srow [512]        sum_l exp(20*(sims_intra - mrow))  (ACT Exp + accum)
  - Host merges: global intra logsumexp from (mrow, srow) pairs; global top-50
    hard negatives from the 8x256 candidates with the label-masked (positive)
    columns removed by value-matching; positives (8 values/row, 0.01% of the
    similarity matrix) are computed on host in f64.
  - Exactness certificate: every 250-chunk's 8th-largest value must be <= the
    50th-largest merged candidate; rows violating it (probability ~1e-11 per
    chunk on this data) are recomputed exactly on host.
"""

import sys
import functools

sys.path.insert(0, "/opt/trn_rl_repo")

import numpy as np

from concourse import bass, bacc, mybir
from concourse.bass_utils import run_bass_kernel_spmd
from concourse.tile import TileContext

F32 = mybir.dt.float32

N = 512          # batch
D = 256          # feature dim
L = 8000         # labels
C = 8            # cameras
P_LOCAL = 8000   # center columns per core (= 1000 labels * 8 cams)
L_LOCAL = 1000   # labels per core
NCORES = 8
RT = 4           # row tiles of 128
INV_T = 20.0     # 1 / temperature
K = 50           # hard negatives
LW = 0.5         # inter-cam loss weight

MM_CHUNK = 500   # matmul moving free dim (PSUM bank = 512 f32)
N_MM = P_LOCAL // MM_CHUNK         # 16
MAX_CHUNK = 250  # top-8 extraction chunk
N_MAXCH = P_LOCAL // MAX_CHUNK     # 32
CAND = N_MAXCH * 8                 # 256 candidate values per row per core

# matmul operand dtype: float32 (exact, PE 4 cyc/row), float32r (PE 1 cyc/row
# at moving dim >= 256), bfloat16 (1 cyc/row + half DMA)
MM_DT = mybir.dt.float32
MM_NP = np.float32


def _block_cams(cam_bounds, rt):
    """For each 32-row block of row-tile rt: cameras present in the block.

    Engine SBUF accesses must start on a 32-partition boundary, so the intra
    reductions run per aligned 32-block, once per camera present, writing to
    that camera's slot of a [128, C] tile; the host picks each row's slot.
    """
    lo = 128 * rt
    out = []
    for b in range(4):
        blo, bhi = lo + 32 * b, lo + 32 * b + 32
        cams = [
            c
            for c in range(C)
            if max(cam_bounds[c], blo) < min(cam_bounds[c + 1], bhi)
        ]
        out.append(cams)
    return out


@functools.lru_cache(maxsize=4)
def _build_program(cam_bounds):
    nc = bacc.Bacc(None, target_bir_lowering=False)

    cenT = nc.dram_tensor("cenT", [2, 128, P_LOCAL], MM_DT, kind="ExternalInput")
    featsd = nc.dram_tensor("feats", [RT, 128, D], F32, kind="ExternalInput")
    identd = nc.dram_tensor("ident", [128, 128], F32, kind="ExternalInput")
    candd = nc.dram_tensor("cand", [RT, 128, CAND], F32, kind="ExternalOutput")
    mrowd = nc.dram_tensor("mrow", [RT, 128, C], F32, kind="ExternalOutput")
    srowd = nc.dram_tensor("srow", [RT, 128, C], F32, kind="ExternalOutput")

    ActF = mybir.ActivationFunctionType
    Axis = mybir.AxisListType

    with TileContext(nc) as tc:
        with (
            tc.tile_pool(name="cen", bufs=1) as cenp,
            tc.tile_pool(name="ftp", bufs=1) as ftp,
            tc.tile_pool(name="simsp", bufs=2) as simsp,
            tc.tile_pool(name="smallp", bufs=2) as smallp,
            tc.tile_pool(name="outp", bufs=2) as outp,
            tc.tile_pool(name="psum", bufs=4, space="PSUM") as psump,
            tc.tile_pool(name="psumT", bufs=2, space="PSUM") as psumtp,
        ):
            # center shard halves, [128 (k), 8000 (j)] each
            cen_sb = []
            for kh in range(2):
                cent = cenp.tile([128, P_LOCAL], MM_DT, name=f"cen{kh}")
                for j in range(4):
                    s = slice(j * 2000, (j + 1) * 2000)
                    nc.sync.dma_start(out=cent[:, s], in_=cenT[kh, :, s])
                cen_sb.append(cent)

            ident_sb = smallp.tile([128, 128], F32, name="ident_sb", bufs=1)
            nc.sync.dma_start(out=ident_sb[:, :], in_=identd[:, :])

            # feats: load, L2-normalize rows, transpose to [k, i] blocks
            fTs = []
            for rt in range(RT):
                ftile = smallp.tile([128, D], F32, name="ftile")
                nc.sync.dma_start(out=ftile[:, :], in_=featsd[rt])
                fsq = smallp.tile([128, D], F32, name="fsq")
                nc.vector.tensor_mul(fsq[:, :], ftile[:, :], ftile[:, :])
                n2 = smallp.tile([128, 1], F32, name="n2")
                nc.vector.reduce_sum(n2[:, :], fsq[:, :], axis=Axis.X)
                nrm = smallp.tile([128, 1], F32, name="nrm")
                nc.scalar.sqrt(nrm[:, :], n2[:, :])
                inv = smallp.tile([128, 1], F32, name="inv")
                nc.vector.reciprocal(inv[:, :], nrm[:, :])
                fn = smallp.tile([128, D], F32, name="fn")
                nc.vector.tensor_scalar_mul(fn[:, :], ftile[:, :], inv[:, 0:1])
                fT0 = ftp.tile([128, 128], MM_DT, name=f"fT{rt}_0")
                fT1 = ftp.tile([128, 128], MM_DT, name=f"fT{rt}_1")
                for kh, fT in ((0, fT0), (1, fT1)):
                    pt = psumtp.tile([128, 128], F32, name="pT")
                    nc.tensor.transpose(
                        pt[:, :], fn[:, kh * 128 : (kh + 1) * 128], ident_sb[:, :]
                    )
                    nc.scalar.copy(fT[:, :], pt[:, :])
                fTs.append((fT0, fT1))

            for rt in range(RT):
                sims = simsp.tile([128, P_LOCAL], F32, name="sims")
                cand_t = outp.tile([128, CAND], F32, name="cand_t")
                for ck in range(N_MM):
                    ps = psump.tile([128, MM_CHUNK], F32, name="ps")
                    s = slice(ck * MM_CHUNK, (ck + 1) * MM_CHUNK)
                    nc.tensor.matmul(
                        ps[:, :], fTs[rt][0][:, :], cen_sb[0][:, s],
                        start=True, stop=False,
                    )
                    nc.tensor.matmul(
                        ps[:, :], fTs[rt][1][:, :], cen_sb[1][:, s],
                        start=False, stop=True,
                    )
                    nc.scalar.copy(sims[:, s], ps[:, :])
                    j0 = ck * MM_CHUNK
                    c0 = ck * 16
                    nc.vector.max(cand_t[:, c0 : c0 + 8], sims[:, j0 : j0 + 250])
                    nc.vector.max(
                        cand_t[:, c0 + 8 : c0 + 16], sims[:, j0 + 250 : j0 + 500]
                    )

                # intra: camera-selected strided views, max + sum(exp),
                # per aligned 32-row block, one slot per camera present
                m_t = smallp.tile([128, C], F32, name="m_t")
                s_t = smallp.tile([128, C], F32, name="s_t")
                bias_t = smallp.tile([128, C], F32, name="bias_t")
                scr = smallp.tile([128, L_LOCAL], F32, name="scr")
                simsr = sims.rearrange("p (l c) -> p l c", c=C)
                blocks = _block_cams(cam_bounds, rt)
                for b, bcams in enumerate(blocks):
                    p0, p1 = 32 * b, 32 * b + 32
                    for cam in bcams:
                        nc.vector.reduce_max(
                            m_t[p0:p1, cam : cam + 1],
                            simsr[p0:p1, :, cam],
                            axis=Axis.X,
                        )
                nc.vector.tensor_scalar_mul(bias_t[:, :], m_t[:, :], -INV_T)
                for b, bcams in enumerate(blocks):
                    p0, p1 = 32 * b, 32 * b + 32
                    for cam in bcams:
                        nc.scalar.activation(
                            scr[p0:p1, :],
                            simsr[p0:p1, :, cam],
                            ActF.Exp,
                            bias=bias_t[p0:p1, cam : cam + 1],
                            scale=INV_T,
                            accum_out=s_t[p0:p1, cam : cam + 1],
                        )
                nc.sync.dma_start(out=candd[rt], in_=cand_t[:, :])
                nc.sync.dma_start(out=mrowd[rt], in_=m_t[:, :])
                nc.sync.dma_start(out=srowd[rt], in_=s_t[:, :])

    nc.compile()
    return nc


class _Runner:
    """Sharded 8-core executor for a built Bass program.

    Builds the jax.jit(shard_map(bass_exec)) executable once (the walrus/NEFF
    compile happens inside the first call) and reuses it for every subsequent
    execution, keeping large inputs device-resident.
    """

    def __init__(self, nc, n_cores=NCORES):
        import jax
        from jax.sharding import Mesh, PartitionSpec, NamedSharding
        from jax.experimental.shard_map import shard_map
        from concourse import bass2jax

        self.jax = jax
        self.nc = nc
        self.n_cores = n_cores
        bass2jax.install_neuronx_cc_hook()
        partition_name = (
            nc.partition_id_tensor.name if nc.partition_id_tensor else None
        )
        in_names, out_names, out_avals = [], [], []
        for alloc in nc.m.functions[0].allocations:
            if not isinstance(alloc, mybir.MemoryLocationSet):
                continue
            name = alloc.memorylocations[0].name
            if alloc.kind == "ExternalInput":
                if name != partition_name:
                    in_names.append(name)
            elif alloc.kind == "ExternalOutput":
                out_names.append(name)
                out_avals.append(
                    jax.core.ShapedArray(
                        tuple(alloc.tensor_shape), mybir.dt.np(alloc.dtype)
                    )
                )
        self.in_names, self.out_names, self.out_avals = in_names, out_names, out_avals
        n_params, n_outs = len(in_names), len(out_avals)
        all_in_names = list(in_names) + list(out_names)
        if partition_name is not None:
            all_in_names.append(partition_name)

        def _body(*args):
            operands = list(args)
            if partition_name is not None:
                operands.append(bass2jax.partition_id_tensor())
            return tuple(
                bass2jax._bass_exec_p.bind(
                    *operands,
                    out_avals=tuple(out_avals),
                    in_names=tuple(all_in_names),
                    out_names=tuple(out_names),
                    lowering_input_output_aliases=(),
                    sim_require_finite=True,
                    sim_require_nnan=True,
                    nc=nc,
                )
            )

        devices = jax.devices()[:n_cores]
        self.mesh = Mesh(np.asarray(devices), ("core",))
        self.sh = NamedSharding(self.mesh, PartitionSpec("core"))
        self.fn = jax.jit(
            shard_map(
                _body,
                mesh=self.mesh,
                in_specs=(PartitionSpec("core"),) * (n_params + n_outs),
                out_specs=(PartitionSpec("core"),) * n_outs,
                check_rep=False,
            ),
            donate_argnums=tuple(range(n_params, n_params + n_outs)),
            keep_unused=True,
        )
        self._zero_shapes = [
            ((n_cores * a.shape[0], *a.shape[1:]), a.dtype) for a in out_avals
        ]

    def put_inputs(self, in_maps):
        self.dev_in = [
            self.jax.device_put(
                np.concatenate([np.asarray(m[name]) for m in in_maps], axis=0),
                self.sh,
            )
            for name in self.in_names
        ]

    def _zeros(self):
        return [
            self.jax.device_put(np.zeros(s, d), self.sh)
            for s, d in self._zero_shapes
        ]

    def execute(self):
        outs = self.fn(*self.dev_in, *self._zeros())
        self.jax.block_until_ready(outs)
        return self.unpack(outs)

    def unpack(self, outs):
        return [
            {
                name: np.asarray(outs[i]).reshape(
                    self.n_cores, *self.out_avals[i].shape
                )[c]
                for i, name in enumerate(self.out_names)
            }
            for c in range(self.n_cores)
        ]


_RUNNERS = {}


def _get_runner(nc):
    r = _RUNNERS.get(id(nc))
    if r is None:
        r = _Runner(nc)
        _RUNNERS[id(nc)] = r
    return r


def _make_in_maps(cenT_shards, feats_p):
    ident = np.eye(128, dtype=np.float32)
    fin = np.ascontiguousarray(feats_p.reshape(RT, 128, D), dtype=np.float32)
    return [
        {
            "cenT": np.ascontiguousarray(
                cenT_shards[k].reshape(2, 128, P_LOCAL), dtype=MM_NP
            ),
            "feats": fin,
            "ident": ident,
        }
        for k in range(NCORES)
    ]


def _host_finish(results, feats_p, labels_p, cams_p, centers):
    cand = np.stack(
        [results[k]["cand"].reshape(N, CAND) for k in range(NCORES)]
    )  # [8, 512, 256]
    rows = np.arange(N)
    m_k = np.stack(
        [results[k]["mrow"].reshape(N, C)[rows, cams_p] for k in range(NCORES)]
    ).astype(np.float64)  # [8, 512]
    s_k = np.stack(
        [results[k]["srow"].reshape(N, C)[rows, cams_p] for k in range(NCORES)]
    ).astype(np.float64)

    fe = feats_p.astype(np.float64)
    fn = fe / np.linalg.norm(fe, axis=1, keepdims=True)
    cen = centers.astype(np.float64)

    # positives: 8 same-label proxies per row (host, f64)
    gidx = labels_p[:, None] * C + np.arange(C)[None, :]        # [512, 8]
    g = cen[gidx]                                               # [512, 8, 256]
    pos = np.einsum("rcd,rd->rc", g, fn)                        # [512, 8]

    # ---- intra ----
    M = m_k.max(axis=0)
    S = (s_k * np.exp(INV_T * (m_k - M))).sum(axis=0)
    lse_intra = np.log(S) + INV_T * M
    v = pos[np.arange(N), cams_p]
    loss_intra_i = lse_intra - INV_T * v

    # ---- inter: merge candidates, remove positive columns by value ----
    CR = cand.transpose(1, 0, 2).reshape(N, NCORES * CAND).astype(np.float64)
    owner = labels_p // L_LOCAL
    lloc = labels_p % L_LOCAL
    col0 = C * lloc                                             # local column of 1st positive
    ch0 = col0 // MAX_CHUNK
    ch1 = (col0 + C - 1) // MAX_CHUNK
    eps = 1e-5
    for i in rows:
        base = owner[i] * CAND
        chunks = {ch0[i], ch1[i]}
        idxs = np.concatenate([np.arange(base + 8 * ch, base + 8 * ch + 8)
                               for ch in sorted(chunks)])
        vals = CR[i, idxs]
        used = np.zeros(len(idxs), bool)
        for pv in pos[i]:
            d = np.abs(vals - pv)
            d[used] = np.inf
            j = np.argmin(d)
            if d[j] < eps:
                used[j] = True
        CR[i, idxs[used]] = -np.inf

    part = np.partition(CR, NCORES * CAND - K, axis=1)[:, -K:]  # top-50 values
    t50 = part.min(axis=1)

    # certificate: every chunk's 8th-largest (pre-removal) must be <= t50
    chunk8 = cand[:, :, 7::8].astype(np.float64)                # [8, 512, 32]
    bad = np.where(chunk8.max(axis=(0, 2)) > t50)[0]
    for i in bad:
        sims_row = cen @ fn[i]                                  # [64000] exact
        sims_row[C * labels_p[i] : C * labels_p[i] + C] = -np.inf
        part[i] = np.sort(sims_row)[-K:]

    z = np.concatenate([pos, part], axis=1) * INV_T             # [512, 58]
    mz = z.max(axis=1)
    lse_inter = np.log(np.exp(z - mz[:, None]).sum(axis=1)) + mz
    loss_inter_i = lse_inter - INV_T * pos.mean(axis=1)

    # ---- per-camera means, summed ----
    cnt = np.bincount(cams_p, minlength=C).astype(np.float64)
    s_intra = np.bincount(cams_p, weights=loss_intra_i, minlength=C)
    s_inter = np.bincount(cams_p, weights=loss_inter_i, minlength=C)
    safe = np.maximum(cnt, 1.0)
    li = np.sum(np.where(cnt > 0, s_intra / safe, 0.0))
    le = LW * np.sum(np.where(cnt > 0, s_inter / safe, 0.0))
    return np.array([li, le], dtype=np.float32)


def kernel(feats, indexes, label_table, cam_table, centers):
    feats = np.asarray(feats, dtype=np.float32)
    indexes = np.asarray(indexes)
    label_table = np.asarray(label_table)
    cam_table = np.asarray(cam_table)
    centers = np.asarray(centers, dtype=np.float32)

    labels = np.asarray(label_table[indexes], dtype=np.int64)
    cams = np.asarray(cam_table[indexes], dtype=np.int64)

    perm = np.argsort(cams, kind="stable")
    feats_p = np.ascontiguousarray(feats[perm])
    labels_p = labels[perm]
    cams_p = cams[perm]
    cam_bounds = tuple(np.searchsorted(cams_p, np.arange(C + 1)).tolist())

    nc = _build_program(cam_bounds)
    cenT_shards = [
        np.ascontiguousarray(centers[k * P_LOCAL : (k + 1) * P_LOCAL].T)
        for k in range(NCORES)
    ]
    runner = _get_runner(nc)
    runner.put_inputs(_make_in_maps(cenT_shards, feats_p))
    results = runner.execute()
    return _host_finish(results, feats_p, labels_p, cams_p, centers)
